# revision 15
# baseline (speedup 1.0000x reference)
"""Causal self-attention for Trainium2, 8 NeuronCores.

Sharding: tensor-parallel over heads (4 heads/core) x data-parallel over
batch (2). Core i handles batch i//4, heads 4*(i%4)..4*(i%4)+3. Each core
computes its heads' attention output and a partial output projection
(W_proj rows for its heads); the host sums the 4 partials per batch and
adds b_proj.

Device layout choices:
  - Q^T, K^T computed feature-major [dim, t] directly (lhsT = W chunk,
    rhs = x^T chunk), so attention scores come out as S^T [k, q] with k
    on partitions -- which is exactly the layout the P@V matmul needs
    as its rhs. No on-chip transposes of the O(T^2) object.
  - V computed in natural [t, dim] layout (lhsT = x^T chunk, rhs = W_v),
    which is the lhsT layout the P@V matmul needs. A ones-column is
    appended to V so the softmax denominators fall out of the same
    matmul (row 64*... of the PSUM output).
  - exp() without max subtraction: scores are q.k/8 with q,k ~ N(0,1),
    bounded well inside fp32 exp range; softmax is shift-invariant so
    the result is mathematically identical to the reference.
  - all matmuls run as float32r (replicated fp32) with free dim >= 256,
    which streams at 1 column/cycle like bf16.

The causal mask is handled by skipping fully-masked k-chunks and
multiplying exp(S) by one of 4 precomputed 0/1 indicator tiles on the
diagonal-straddling chunks. If the runtime mask is not the lower-tri
causal mask, a general fallback multiplies by the actual mask (DMA'd
transposed) instead; an all-ones mask drops masking entirely.
"""

import numpy as np

B, T, C, H = 2, 2048, 1024, 16
D = C // H            # 64 head dim
NCORES = 8
NBG = 2               # batch shards
NHG = 4               # head-group shards
HL = H // NHG         # 4 heads per core
DL = HL * D           # 256 local feature dims
NDQ = DL // 128       # 2 partition chunks of local dims
NTB = T // 512        # 4 t-chunks of 512
NKC = T // 128        # 16 key chunks of 128
NQC = T // 512        # 4 query chunks of 512
NTT = T // 128        # 16 t-tiles of 128 (proj / V)

_CACHE = {}


def _build(mode, debug_dump=False):
    """Build + compile the per-core Bass program. mode: causal|full|general."""
    import concourse.bacc as bacc
    import concourse.tile as tile
    import concourse.mybir as mybir

    f32 = mybir.dt.float32
    bf16 = mybir.dt.bfloat16
    Exp = mybir.ActivationFunctionType.Exp
    Ident = mybir.ActivationFunctionType.Identity
    mult = mybir.AluOpType.mult
    add = mybir.AluOpType.add

    nc = bacc.Bacc(
        "TRN2", target_bir_lowering=False, debug=False, num_devices=NCORES
    )

    xT = nc.dram_tensor("xT", [C, T], bf16, kind="ExternalInput").ap()
    Wl = nc.dram_tensor("Wl", [C, 3 * DL], bf16, kind="ExternalInput").ap()
    bqk = nc.dram_tensor("bqk", [128, 2 * NDQ], f32, kind="ExternalInput").ap()
    bv = nc.dram_tensor("bv", [1, DL], f32, kind="ExternalInput").ap()
    Wp = nc.dram_tensor("Wp", [DL, C], bf16, kind="ExternalInput").ap()
    maskT = None
    if mode == "general":
        maskT = nc.dram_tensor("maskT", [T, T], bf16, kind="ExternalInput").ap()
    yp = nc.dram_tensor("yp", [T, C], f32, kind="ExternalOutput").ap()
    dbg = {}
    if debug_dump:
        for nm, shp, dt in [
            ("qt_d", [128, NDQ, T], bf16), ("kt_d", [128, NDQ, T], bf16),
            ("v1_d", [128, NKC, HL, D + 1], bf16), ("ot_d", [128, NDQ, T], bf16),
            ("st_d", [128, 512], f32), ("p_d", [128, 512], bf16),
            ("o_d", [65, 512], f32),
        ]:
            dbg[nm] = nc.dram_tensor(nm, shp, dt, kind="ExternalOutput").ap()

    with tile.TileContext(nc) as tc:
        with (
            tc.tile_pool(name="singles", bufs=1) as singles,
            tc.tile_pool(name="xin", bufs=2) as xin,
            tc.tile_pool(name="ptiles", bufs=6) as ptiles,
            tc.tile_pool(name="small", bufs=4) as small,
            tc.tile_pool(name="outp", bufs=3) as outp,
            tc.tile_pool(name="psum", bufs=7, space="PSUM") as psum,
        ):
            def ps512(name):
                return psum.tile([128, 512], f32, name="ps512", tag="ps512")

            # ---- resident inputs ----
            W_sb = singles.tile([128, 8, 3 * DL], bf16)
            nc.sync.dma_start(
                out=W_sb, in_=Wl.rearrange("(kc p) n -> p kc n", p=128)
            )
            bqk_sb = singles.tile([128, 2 * NDQ], f32)
            nc.sync.dma_start(out=bqk_sb, in_=bqk)
            bv_row = singles.tile([1, DL], f32)
            nc.sync.dma_start(out=bv_row, in_=bv)
            bv_sb = singles.tile([128, DL], f32)
            nc.gpsimd.partition_broadcast(bv_sb, bv_row)
            Wp_sb = singles.tile([128, NDQ, C], bf16)
            nc.sync.dma_start(
                out=Wp_sb, in_=Wp.rearrange("(dq p) n -> p dq n", p=128)
            )

            ind = None
            if mode == "causal":
                ind = singles.tile([128, 4, 512], bf16)
                for j in range(4):
                    nc.vector.memset(ind[:, j, :], 1.0)
                    # keep (=1.0) iff f - p - 128*j >= 0, else 0.0
                    nc.gpsimd.affine_select(
                        out=ind[:, j, :],
                        in_=ind[:, j, :],
                        compare_op=mybir.AluOpType.is_ge,
                        fill=0.0,
                        base=-128 * j,
                        pattern=[[1, 512]],
                        channel_multiplier=-1,
                    )

            # ---- resident intermediates ----
            QT = singles.tile([128, NDQ, T], bf16)   # [dim%128, dimchunk, t]
            KT = singles.tile([128, NDQ, T], bf16)
            V1 = singles.tile([128, NKC, HL, D + 1], bf16)  # [t%128, kc, h, d+1]
            nc.vector.memset(V1[:, :, :, D : D + 1], 1.0)
            OT = singles.tile([128, NDQ, T], bf16)

            # ---- phase 1: QKV projections ----
            for tb in range(NTB):
                x_sb = xin.tile([128, 8, 512], bf16)
                nc.sync.dma_start(
                    out=x_sb,
                    in_=xT.rearrange("(kc p) t -> p kc t", p=128)[
                        :, :, tb * 512 : (tb + 1) * 512
                    ],
                )
                # Q^T and K^T, feature-major
                for s in range(2):  # 0=Q, 1=K
                    for dq in range(NDQ):
                        ps = ps512("qk")
                        col = s * DL + dq * 128
                        for kc in range(8):
                            nc.tensor.matmul(
                                ps,
                                lhsT=W_sb[:, kc, col : col + 128],
                                rhs=x_sb[:, kc, :],
                                start=(kc == 0),
                                stop=(kc == 7),
                            )
                        dst = (QT if s == 0 else KT)[
                            :, dq, tb * 512 : (tb + 1) * 512
                        ]
                        nc.scalar.activation(
                            dst, ps, Ident,
                            bias=bqk_sb[:, s * NDQ + dq : s * NDQ + dq + 1],
                        )
                # V natural [t, d] with bias, into V1
                for t4 in range(4):
                    tt = tb * 4 + t4
                    ps = ps512("v")
                    for kc in range(8):
                        nc.tensor.matmul(
                            ps[:, :DL],
                            lhsT=x_sb[:, kc, t4 * 128 : (t4 + 1) * 128],
                            rhs=W_sb[:, kc, 2 * DL : 3 * DL],
                            start=(kc == 0),
                            stop=(kc == 7),
                        )
                    nc.vector.tensor_tensor(
                        out=V1[:, tt, :, 0:D],
                        in0=ps[:, :DL].rearrange("p (h d) -> p h d", d=D),
                        in1=bv_sb.rearrange("p (h d) -> p h d", d=D),
                        op=add,
                    )

            if debug_dump:
                nc.sync.dma_start(out=dbg["qt_d"], in_=QT)
                nc.sync.dma_start(out=dbg["kt_d"], in_=KT)
                nc.sync.dma_start(out=dbg["v1_d"], in_=V1)

            # ---- phase 2: attention, S^T layout [k, q] ----
            for qc in range(NQC):
                nkc = 4 * qc + 4 if mode == "causal" else NKC
                m_sb = None
                if mode == "general":
                    m_sb = xin.tile([128, NKC, 512], bf16, tag="mask", bufs=1)
                    nc.sync.dma_start(
                        out=m_sb,
                        in_=maskT.rearrange("(kc p) q -> p kc q", p=128)[
                            :, :, qc * 512 : (qc + 1) * 512
                        ],
                    )
                for hp in range(NDQ):  # head pair = partition chunk
                    ops = [ps512("o"), ps512("o")]  # per head in pair
                    for kc in range(nkc):
                        for hh in range(2):
                            h = hp * 2 + hh
                            off = 64 * hh
                            st = ps512("st")
                            nc.tensor.matmul(
                                st,
                                lhsT=KT[
                                    off : off + 64, hp, kc * 128 : (kc + 1) * 128
                                ],
                                rhs=QT[
                                    off : off + 64, hp, qc * 512 : (qc + 1) * 512
                                ],
                                start=True,
                                stop=True,
                            )
                            p_sb = ptiles.tile([128, 512], bf16, tag="p")
                            if debug_dump and hp == 0 and qc == 0 and kc == 0 and hh == 0:
                                st_cp = ptiles.tile([128, 512], f32, tag="stcp")
                                nc.vector.tensor_copy(st_cp, st)
                                nc.sync.dma_start(out=dbg["st_d"], in_=st_cp)
                            nc.scalar.activation(p_sb, st, Exp)
                            if mode == "causal" and kc >= 4 * qc:
                                nc.vector.tensor_tensor(
                                    out=p_sb, in0=p_sb,
                                    in1=ind[:, kc - 4 * qc, :], op=mult,
                                )
                            elif mode == "general":
                                nc.vector.tensor_tensor(
                                    out=p_sb, in0=p_sb, in1=m_sb[:, kc, :], op=mult
                                )
                            if debug_dump and hp == 0 and qc == 0 and kc == 0 and hh == 0:
                                nc.sync.dma_start(out=dbg["p_d"], in_=p_sb)
                            nc.tensor.matmul(
                                ops[hh][: D + 1, :],
                                lhsT=V1[:, kc, h, :],
                                rhs=p_sb,
                                start=(kc == 0),
                                stop=(kc == nkc - 1),
                            )
                    if debug_dump and hp == 0 and qc == 0:
                        o_cp = ptiles.tile([65, 512], f32, tag="ocp")
                        nc.vector.tensor_copy(o_cp, ops[0][: D + 1, :])
                        nc.sync.dma_start(out=dbg["o_d"], in_=o_cp)
                    for hh in range(2):
                        off = 64 * hh
                        rcp = small.tile([1, 512], f32, tag="rcp")
                        nc.vector.reciprocal(rcp, ops[hh][D : D + 1, :])
                        rb = small.tile([64, 512], f32, tag="rb")
                        nc.gpsimd.partition_broadcast(rb, rcp)
                        nc.vector.tensor_tensor(
                            out=OT[off : off + 64, hp, qc * 512 : (qc + 1) * 512],
                            in0=ops[hh][0:D, :],
                            in1=rb,
                            op=mult,
                        )

            if debug_dump:
                nc.sync.dma_start(out=dbg["ot_d"], in_=OT)

            # ---- phase 3: partial output projection ----
            for tt in range(NTT):
                y_sb = outp.tile([128, C], f32)
                for n in range(2):
                    pp = ps512("proj")
                    for dq in range(NDQ):
                        nc.tensor.matmul(
                            pp,
                            lhsT=OT[:, dq, tt * 128 : (tt + 1) * 128],
                            rhs=Wp_sb[:, dq, n * 512 : (n + 1) * 512],
                            start=(dq == 0),
                            stop=(dq == NDQ - 1),
                        )
                    nc.vector.tensor_copy(y_sb[:, n * 512 : (n + 1) * 512], pp)
                nc.sync.dma_start(
                    out=yp[tt * 128 : (tt + 1) * 128, :], in_=y_sb
                )

    nc.compile()
    return nc


def _host_prep(x, prefix_causal_mask, W_attn, b_attn, W_proj):
    """Split full inputs into 8 per-core input maps; detect mask mode."""
    scale = 1.0 / np.sqrt(np.float32(D))
    mask = np.asarray(prefix_causal_mask)
    if mask.all():
        mode = "full"
    else:
        tri = np.tril(np.ones((T, T), dtype=bool))
        if all(np.array_equal(mask[b], tri) for b in range(B)):
            mode = "causal"
        else:
            mode = "general"

    import ml_dtypes

    bf16 = ml_dtypes.bfloat16
    x = np.asarray(x, dtype=np.float32)
    W_attn = np.asarray(W_attn, dtype=np.float32)
    b_attn = np.asarray(b_attn, dtype=np.float32)
    W_proj = np.asarray(W_proj, dtype=np.float32)

    in_maps = []
    for core in range(NCORES):
        b = core // NHG
        hg = core % NHG
        lo = hg * DL
        hi = lo + DL
        xT = np.ascontiguousarray(x[b].T)  # [C, T]
        Wq = W_attn[:, lo:hi] * scale
        Wk = W_attn[:, C + lo : C + hi]
        Wv = W_attn[:, 2 * C + lo : 2 * C + hi]
        Wl = np.ascontiguousarray(np.concatenate([Wq, Wk, Wv], axis=1))
        bq = b_attn[lo:hi] * scale
        bk = b_attn[C + lo : C + hi]
        # bias per partition for Q,K chunks: cols = [q0, q1, k0, k1]
        bqk = np.stack(
            [bq[0:128], bq[128:256], bk[0:128], bk[128:256]], axis=1
        ).astype(np.float32)
        bv = np.ascontiguousarray(
            b_attn[2 * C + lo : 2 * C + hi][None, :]
        ).astype(np.float32)
        Wp = np.ascontiguousarray(W_proj[lo:hi, :])
        im = {
            "xT": xT.astype(bf16),
            "Wl": Wl.astype(bf16),
            "bqk": np.ascontiguousarray(bqk),
            "bv": bv,
            "Wp": Wp.astype(bf16),
        }
        if mode == "general":
            im["maskT"] = np.ascontiguousarray(mask[b].T).astype(bf16)
        in_maps.append(im)
    return mode, in_maps


def _get_program(mode):
    if mode not in _CACHE:
        _CACHE[mode] = _build(mode)
    return _CACHE[mode]


def _run(inputs, trace=False):
    """Returns (full_output [B,T,C], BassKernelResults)."""
    from concourse import bass_utils

    mode, in_maps = _host_prep(
        inputs["x"],
        inputs["prefix_causal_mask"],
        inputs["W_attn"],
        inputs["b_attn"],
        inputs["W_proj"],
    )
    nc = _get_program(mode)
    res = bass_utils.run_bass_kernel_spmd(
        nc, in_maps, core_ids=list(range(NCORES)), trace=trace
    )
    b_proj = np.asarray(inputs["b_proj"], dtype=np.float32)
    y = np.zeros((B, T, C), dtype=np.float32)
    for core in range(NCORES):
        y[core // NHG] += res.results[core]["yp"]
    y += b_proj[None, None, :]
    return y, res


def kernel(**inputs):
    y, _ = _run(inputs, trace=False)
    return y


# revision 25
# speedup vs baseline: 1.1637x; 1.1637x over previous
"""Causal self-attention for Trainium2, 8 NeuronCores.

Sharding: tensor-parallel over heads (4 heads/core) x data-parallel over
batch (2). Core i handles batch i//4, heads 4*(i%4)..4*(i%4)+3. Each core
computes its heads' attention output and a partial output projection
(W_proj rows for its heads); the host sums the 4 partials per batch and
adds b_proj.

Device layout choices:
  - Q^T, K^T computed feature-major [dim, t] directly (lhsT = W chunk,
    rhs = x^T chunk), so attention scores come out as S^T [k, q] with k
    on partitions -- which is exactly the layout the P@V matmul needs
    as its rhs. No on-chip transposes of the O(T^2) object.
  - V computed in natural [t, dim] layout (lhsT = x^T chunk, rhs = W_v),
    which is the lhsT layout the P@V matmul needs. A ones-column is
    appended to V so the softmax denominators fall out of the same
    matmul (row 64*... of the PSUM output).
  - exp() without max subtraction: scores are q.k/8 with q,k ~ N(0,1),
    bounded well inside fp32 exp range; softmax is shift-invariant so
    the result is mathematically identical to the reference.
  - all matmuls run as float32r (replicated fp32) with free dim >= 256,
    which streams at 1 column/cycle like bf16.

The causal mask is handled by skipping fully-masked k-chunks and
multiplying exp(S) by one of 4 precomputed 0/1 indicator tiles on the
diagonal-straddling chunks. If the runtime mask is not the lower-tri
causal mask, a general fallback multiplies by the actual mask (DMA'd
transposed) instead; an all-ones mask drops masking entirely.
"""

import numpy as np

B, T, C, H = 2, 2048, 1024, 16
D = C // H            # 64 head dim
NCORES = 8
NBG = 2               # batch shards
NHG = 4               # head-group shards
HL = H // NHG         # 4 heads per core
DL = HL * D           # 256 local feature dims
NDQ = DL // 128       # 2 partition chunks of local dims
NTB = T // 512        # 4 t-chunks of 512
NKC = T // 128        # 16 key chunks of 128
NQC = T // 512        # 4 query chunks of 512
NTT = T // 128        # 16 t-tiles of 128 (proj / V)

_CACHE = {}


def _build(mode, debug_dump=False):
    """Build + compile the per-core Bass program. mode: causal|full|general."""
    import concourse.bass as bass
    import concourse.bacc as bacc
    import concourse.tile as tile
    import concourse.mybir as mybir

    f32 = mybir.dt.float32
    bf16 = mybir.dt.bfloat16
    Exp = mybir.ActivationFunctionType.Exp
    Ident = mybir.ActivationFunctionType.Identity
    mult = mybir.AluOpType.mult
    add = mybir.AluOpType.add

    nc = bacc.Bacc(
        "TRN2", target_bir_lowering=False, debug=False, num_devices=NCORES
    )

    xT = nc.dram_tensor("xT", [C, T], bf16, kind="ExternalInput").ap()
    Wl = nc.dram_tensor("Wl", [C, 3 * DL], bf16, kind="ExternalInput").ap()
    bqk = nc.dram_tensor("bqk", [128, 2 * NDQ], f32, kind="ExternalInput").ap()
    bv = nc.dram_tensor("bv", [1, DL], f32, kind="ExternalInput").ap()
    Wp = nc.dram_tensor("Wp", [DL, C], bf16, kind="ExternalInput").ap()
    maskT = None
    if mode == "general":
        maskT = nc.dram_tensor("maskT", [T, T], bf16, kind="ExternalInput").ap()
    yp = nc.dram_tensor("yp", [T, C], f32, kind="ExternalOutput").ap()
    dbg = {}
    if debug_dump:
        for nm, shp, dt in [
            ("qt_d", [128, NDQ, T], bf16), ("kt_d", [128, NDQ, T], bf16),
            ("v1_d", [128, NKC, HL, D + 1], bf16), ("ot_d", [128, NDQ, T], bf16),
            ("st_d", [128, 512], f32), ("p_d", [128, 512], bf16),
            ("o_d", [65, 512], f32),
        ]:
            dbg[nm] = nc.dram_tensor(nm, shp, dt, kind="ExternalOutput").ap()

    with tile.TileContext(nc) as tc:
        with (
            tc.tile_pool(name="singles", bufs=1) as singles,
            tc.tile_pool(name="xin", bufs=2) as xin,
            tc.tile_pool(name="ptiles", bufs=6) as ptiles,
            tc.tile_pool(name="small", bufs=4) as small,
            tc.tile_pool(name="outp", bufs=3) as outp,
            tc.tile_pool(name="psum", bufs=7, space="PSUM") as psum,
        ):
            def ps512(name):
                return psum.tile([128, 512], f32, name="ps512", tag="ps512")

            # ---- resident inputs ----
            # W and x loads split per kc-chunk so the first matmuls can
            # start as soon as their chunk lands.
            W_sb = singles.tile([128, 8, 3 * DL], bf16)
            Wl_r = Wl.rearrange("(kc p) n -> p kc n", p=128)
            for kc in range(8):
                nc.sync.dma_start(out=W_sb[:, kc, :], in_=Wl_r[:, kc, :])
            bqk_sb = singles.tile([128, 2 * NDQ], f32)
            nc.sync.dma_start(out=bqk_sb, in_=bqk)
            bv_row = singles.tile([1, DL], f32)
            nc.sync.dma_start(out=bv_row, in_=bv)
            bv_sb = singles.tile([128, DL], f32)
            nc.gpsimd.partition_broadcast(bv_sb, bv_row)

            ind = None
            if mode == "causal":
                ind = singles.tile([128, 4, 512], bf16)
                for j in range(4):
                    nc.vector.memset(ind[:, j, :], 1.0)
                    # keep (=1.0) iff f - p - 128*j >= 0, else 0.0
                    nc.gpsimd.affine_select(
                        out=ind[:, j, :],
                        in_=ind[:, j, :],
                        compare_op=mybir.AluOpType.is_ge,
                        fill=0.0,
                        base=-128 * j,
                        pattern=[[1, 512]],
                        channel_multiplier=-1,
                    )

            # ---- resident intermediates ----
            QT = singles.tile([128, NDQ, T], bf16)   # [dim%128, dimchunk, t]
            KT = singles.tile([128, NDQ, T], bf16)
            V1 = singles.tile([128, NKC, HL, D + 1], bf16)  # [t%128, kc, h, d+1]
            nc.vector.memset(V1[:, :, :, D : D + 1], 1.0)
            OT = singles.tile([128, NDQ, T], bf16)
            stage_sb = singles.tile([65, HL * NQC, 512], f32)
            Wp_sb = singles.tile([128, NDQ, C], bf16)
            nc.sync.dma_start(
                out=Wp_sb, in_=Wp.rearrange("(dq p) n -> p dq n", p=128)
            )

            # ---- phase 1: QKV projections ----
            for tb in range(NTB):
                x_sb = xin.tile([128, 8, 512], bf16)
                xr = xT.rearrange("(kc p) t -> p kc t", p=128)[
                    :, :, tb * 512 : (tb + 1) * 512
                ]
                for kc in range(8):
                    nc.sync.dma_start(out=x_sb[:, kc, :], in_=xr[:, kc, :])
                # Q^T and K^T, feature-major
                for s in range(2):  # 0=Q, 1=K
                    for dq in range(NDQ):
                        ps = ps512("qk")
                        col = s * DL + dq * 128
                        for kc in range(8):
                            nc.tensor.matmul(
                                ps,
                                lhsT=W_sb[:, kc, col : col + 128],
                                rhs=x_sb[:, kc, :],
                                start=(kc == 0),
                                stop=(kc == 7),
                            )
                        dst = (QT if s == 0 else KT)[
                            :, dq, tb * 512 : (tb + 1) * 512
                        ]
                        nc.vector.tensor_scalar_add(
                            dst, ps, bqk_sb[:, s * NDQ + dq : s * NDQ + dq + 1]
                        )
                # V natural [t, d] with bias, into V1
                for t4 in range(4):
                    tt = tb * 4 + t4
                    ps = ps512("v")
                    for kc in range(8):
                        nc.tensor.matmul(
                            ps[:, :DL],
                            lhsT=x_sb[:, kc, t4 * 128 : (t4 + 1) * 128],
                            rhs=W_sb[:, kc, 2 * DL : 3 * DL],
                            start=(kc == 0),
                            stop=(kc == 7),
                        )
                    nc.vector.tensor_tensor(
                        out=V1[:, tt, :, 0:D],
                        in0=ps[:, :DL].rearrange("p (h d) -> p h d", d=D),
                        in1=bv_sb.rearrange("p (h d) -> p h d", d=D),
                        op=add,
                    )

            if debug_dump:
                nc.sync.dma_start(out=dbg["qt_d"], in_=QT)
                nc.sync.dma_start(out=dbg["kt_d"], in_=KT)
                nc.sync.dma_start(out=dbg["v1_d"], in_=V1)

            # ---- phase 2: attention, S^T layout [k, q] ----
            for qc in range(NQC):
                nkc = 4 * qc + 4 if mode == "causal" else NKC
                m_sb = None
                if mode == "general":
                    m_sb = xin.tile([128, NKC, 512], bf16, tag="mask", bufs=1)
                    nc.sync.dma_start(
                        out=m_sb,
                        in_=maskT.rearrange("(kc p) q -> p kc q", p=128)[
                            :, :, qc * 512 : (qc + 1) * 512
                        ],
                    )
                for hp in range(NDQ):  # head pair = partition chunk
                    ops = [ps512("o"), ps512("o")]  # per head in pair
                    st_store = {}

                    def emit_mm1(j, hp=hp):
                        pair = []
                        for hh in range(2):
                            off = 64 * hh
                            st = ps512("st")
                            nc.tensor.matmul(
                                st,
                                lhsT=KT[
                                    off : off + 64, hp, j * 128 : (j + 1) * 128
                                ],
                                rhs=QT[
                                    off : off + 64, hp, qc * 512 : (qc + 1) * 512
                                ],
                                start=True,
                                stop=True,
                            )
                            pair.append(st)
                        st_store[j] = pair

                    LOOK = 1  # kc-iterations of MM1 lookahead (PSUM-limited)
                    emitted = min(LOOK + 1, nkc)
                    for j in range(emitted):
                        emit_mm1(j)
                    for kc in range(nkc):
                        pair = st_store.pop(kc)
                        for hh in range(2):
                            h = hp * 2 + hh
                            st = pair[hh]
                            p_sb = ptiles.tile([128, 512], bf16, tag="p")
                            nc.scalar.activation(p_sb, st, Exp)
                            if mode == "causal" and kc >= 4 * qc:
                                nc.vector.tensor_tensor(
                                    out=p_sb, in0=p_sb,
                                    in1=ind[:, kc - 4 * qc, :], op=mult,
                                )
                            elif mode == "general":
                                nc.vector.tensor_tensor(
                                    out=p_sb, in0=p_sb, in1=m_sb[:, kc, :], op=mult
                                )
                            nc.tensor.matmul(
                                ops[hh][: D + 1, :],
                                lhsT=V1[:, kc, h, :],
                                rhs=p_sb,
                                start=(kc == 0),
                                stop=(kc == nkc - 1),
                            )
                        if emitted < nkc:
                            emit_mm1(emitted)
                            emitted += 1
                    # stash unnormalized output rows + softmax denominators
                    # (denominator row stays on partition 64 -- engines can't
                    # move data across partitions; DMA gathers them later)
                    for hh in range(2):
                        h = hp * 2 + hh
                        off = 64 * hh
                        nc.vector.tensor_copy(
                            OT[off : off + 64, hp, qc * 512 : (qc + 1) * 512],
                            ops[hh][0:D, :],
                        )
                        nc.vector.tensor_copy(
                            stage_sb[64:65, h * NQC + qc, :],
                            ops[hh][D : D + 1, :],
                        )

            # batched normalization: gather all 16 denominator rows across
            # partitions (DMA), one reciprocal, DMA-broadcast into OT's
            # layout, one big multiply.
            sums_sb = singles.tile([HL * NQC, 512], f32)
            nc.gpsimd.dma_start(out=sums_sb, in_=stage_sb[64:65, :, :])
            rcp_sb = singles.tile([HL * NQC, 512], f32)
            nc.vector.reciprocal(rcp_sb, sums_sb)
            rcpb_sb = singles.tile([HL * NQC, 512], bf16)
            nc.vector.tensor_copy(rcpb_sb, rcp_sb)
            # SBUF sources can't have partition-step 0; bounce through DRAM
            # so the broadcast reads DRAM with a step-0 partition dim.
            rcp_dram = nc.dram_tensor(
                "rcp_scratch", [HL * NQC, 512], bf16, kind="Internal"
            ).ap()
            nc.sync.dma_start(out=rcp_dram, in_=rcpb_sb)
            rb_full = singles.tile([128, NDQ, T], bf16)
            for h in range(HL):
                hp, off = h // 2, 64 * (h % 2)
                for qc in range(NQC):
                    src = rcp_dram[h * NQC + qc : h * NQC + qc + 1, :]
                    src = bass.AP(
                        tensor=src.tensor,
                        offset=src.offset,
                        ap=[[0, 64], src.ap[1]],
                    )
                    nc.gpsimd.dma_start(
                        out=rb_full[off : off + 64, hp, qc * 512 : (qc + 1) * 512],
                        in_=src,
                    )
            nc.vector.tensor_tensor(out=OT, in0=OT, in1=rb_full, op=mult)

            if debug_dump:
                nc.sync.dma_start(out=dbg["ot_d"], in_=OT)

            # ---- phase 3: partial output projection ----
            for tt in range(NTT):
                y_sb = outp.tile([128, C], f32)
                for n in range(2):
                    pp = ps512("proj")
                    for dq in range(NDQ):
                        nc.tensor.matmul(
                            pp,
                            lhsT=OT[:, dq, tt * 128 : (tt + 1) * 128],
                            rhs=Wp_sb[:, dq, n * 512 : (n + 1) * 512],
                            start=(dq == 0),
                            stop=(dq == NDQ - 1),
                        )
                    nc.vector.tensor_copy(y_sb[:, n * 512 : (n + 1) * 512], pp)
                nc.sync.dma_start(
                    out=yp[tt * 128 : (tt + 1) * 128, :], in_=y_sb
                )

    nc.compile()
    return nc


def _host_prep(x, prefix_causal_mask, W_attn, b_attn, W_proj):
    """Split full inputs into 8 per-core input maps; detect mask mode."""
    scale = 1.0 / np.sqrt(np.float32(D))
    mask = np.asarray(prefix_causal_mask)
    if mask.all():
        mode = "full"
    else:
        tri = np.tril(np.ones((T, T), dtype=bool))
        if all(np.array_equal(mask[b], tri) for b in range(B)):
            mode = "causal"
        else:
            mode = "general"

    import ml_dtypes

    bf16 = ml_dtypes.bfloat16
    x = np.asarray(x, dtype=np.float32)
    W_attn = np.asarray(W_attn, dtype=np.float32)
    b_attn = np.asarray(b_attn, dtype=np.float32)
    W_proj = np.asarray(W_proj, dtype=np.float32)

    in_maps = []
    for core in range(NCORES):
        b = core // NHG
        hg = core % NHG
        lo = hg * DL
        hi = lo + DL
        xT = np.ascontiguousarray(x[b].T)  # [C, T]
        Wq = W_attn[:, lo:hi] * scale
        Wk = W_attn[:, C + lo : C + hi]
        Wv = W_attn[:, 2 * C + lo : 2 * C + hi]
        Wl = np.ascontiguousarray(np.concatenate([Wq, Wk, Wv], axis=1))
        bq = b_attn[lo:hi] * scale
        bk = b_attn[C + lo : C + hi]
        # bias per partition for Q,K chunks: cols = [q0, q1, k0, k1]
        bqk = np.stack(
            [bq[0:128], bq[128:256], bk[0:128], bk[128:256]], axis=1
        ).astype(np.float32)
        bv = np.ascontiguousarray(
            b_attn[2 * C + lo : 2 * C + hi][None, :]
        ).astype(np.float32)
        Wp = np.ascontiguousarray(W_proj[lo:hi, :])
        im = {
            "xT": xT.astype(bf16),
            "Wl": Wl.astype(bf16),
            "bqk": np.ascontiguousarray(bqk),
            "bv": bv,
            "Wp": Wp.astype(bf16),
        }
        if mode == "general":
            im["maskT"] = np.ascontiguousarray(mask[b].T).astype(bf16)
        in_maps.append(im)
    return mode, in_maps


def _get_program(mode):
    if mode not in _CACHE:
        _CACHE[mode] = _build(mode)
    return _CACHE[mode]


def _run(inputs, trace=False):
    """Returns (full_output [B,T,C], BassKernelResults)."""
    from concourse import bass_utils

    mode, in_maps = _host_prep(
        inputs["x"],
        inputs["prefix_causal_mask"],
        inputs["W_attn"],
        inputs["b_attn"],
        inputs["W_proj"],
    )
    nc = _get_program(mode)
    res = bass_utils.run_bass_kernel_spmd(
        nc, in_maps, core_ids=list(range(NCORES)), trace=trace
    )
    b_proj = np.asarray(inputs["b_proj"], dtype=np.float32)
    y = np.zeros((B, T, C), dtype=np.float32)
    for core in range(NCORES):
        y[core // NHG] += res.results[core]["yp"]
    y += b_proj[None, None, :]
    return y, res


def kernel(**inputs):
    y, _ = _run(inputs, trace=False)
    return y


# revision 29
# speedup vs baseline: 1.2863x; 1.1054x over previous
"""Causal self-attention for Trainium2, 8 NeuronCores.

Sharding: tensor-parallel over heads (4 heads/core) x data-parallel over
batch (2). Core i handles batch i//4, heads 4*(i%4)..4*(i%4)+3. Each core
computes its heads' attention output and a partial output projection
(W_proj rows for its heads); the host sums the 4 partials per batch and
adds b_proj.

Device layout choices:
  - Q^T, K^T computed feature-major [dim, t] directly (lhsT = W chunk,
    rhs = x^T chunk), so attention scores come out as S^T [k, q] with k
    on partitions -- which is exactly the layout the P@V matmul needs
    as its rhs. No on-chip transposes of the O(T^2) object.
  - V computed in natural [t, dim] layout (lhsT = x^T chunk, rhs = W_v),
    which is the lhsT layout the P@V matmul needs. A ones-column is
    appended to V so the softmax denominators fall out of the same
    matmul (row 64*... of the PSUM output).
  - exp() without max subtraction: scores are q.k/8 with q,k ~ N(0,1),
    bounded well inside fp32 exp range; softmax is shift-invariant so
    the result is mathematically identical to the reference.
  - all matmuls run as float32r (replicated fp32) with free dim >= 256,
    which streams at 1 column/cycle like bf16.

The causal mask is handled by skipping fully-masked k-chunks and
multiplying exp(S) by one of 4 precomputed 0/1 indicator tiles on the
diagonal-straddling chunks. If the runtime mask is not the lower-tri
causal mask, a general fallback multiplies by the actual mask (DMA'd
transposed) instead; an all-ones mask drops masking entirely.
"""

import numpy as np

B, T, C, H = 2, 2048, 1024, 16
D = C // H            # 64 head dim
NCORES = 8
NBG = 2               # batch shards
NHG = 4               # head-group shards
HL = H // NHG         # 4 heads per core
DL = HL * D           # 256 local feature dims
NDQ = DL // 128       # 2 partition chunks of local dims
NTB = T // 512        # 4 t-chunks of 512
NKC = T // 128        # 16 key chunks of 128
NQC = T // 512        # 4 query chunks of 512
NTT = T // 128        # 16 t-tiles of 128 (proj / V)

_CACHE = {}


def _build(mode, debug_dump=False):
    """Build + compile the per-core Bass program. mode: causal|full|general."""
    import concourse.bass as bass
    import concourse.bacc as bacc
    import concourse.tile as tile
    import concourse.mybir as mybir

    f32 = mybir.dt.float32
    bf16 = mybir.dt.bfloat16
    Exp = mybir.ActivationFunctionType.Exp
    Ident = mybir.ActivationFunctionType.Identity
    mult = mybir.AluOpType.mult
    add = mybir.AluOpType.add

    nc = bacc.Bacc(
        "TRN2", target_bir_lowering=False, debug=False, num_devices=NCORES
    )

    xT = nc.dram_tensor("xT", [C, T], bf16, kind="ExternalInput").ap()
    Wl = nc.dram_tensor("Wl", [C, 3 * DL], bf16, kind="ExternalInput").ap()
    bqk = nc.dram_tensor("bqk", [128, 2 * NDQ], f32, kind="ExternalInput").ap()
    bv = nc.dram_tensor("bv", [1, DL], f32, kind="ExternalInput").ap()
    Wp = nc.dram_tensor("Wp", [DL, C], bf16, kind="ExternalInput").ap()
    maskT = None
    if mode == "general":
        maskT = nc.dram_tensor("maskT", [T, T], bf16, kind="ExternalInput").ap()
    yp = nc.dram_tensor("yp", [T, C], f32, kind="ExternalOutput").ap()
    dbg = {}
    if debug_dump:
        for nm, shp, dt in [
            ("qt_d", [128, NDQ, T], bf16), ("kt_d", [128, NDQ, T], bf16),
            ("v1_d", [128, NKC, HL, D + 1], bf16), ("ot_d", [128, NDQ, T], bf16),
            ("st_d", [128, 512], f32), ("p_d", [128, 512], bf16),
            ("o_d", [65, 512], f32),
        ]:
            dbg[nm] = nc.dram_tensor(nm, shp, dt, kind="ExternalOutput").ap()

    with tile.TileContext(nc) as tc:
        with (
            tc.tile_pool(name="singles", bufs=1) as singles,
            tc.tile_pool(name="xin", bufs=2) as xin,
            tc.tile_pool(name="ptiles", bufs=6) as ptiles,
            tc.tile_pool(name="small", bufs=4) as small,
            tc.tile_pool(name="outp", bufs=3) as outp,
            tc.tile_pool(name="psum", bufs=7, space="PSUM") as psum,
        ):
            def ps512(name):
                return psum.tile(
                    [128, 512], f32, name="ps512", tag="ps512", bufs=4
                )

            # ---- resident inputs ----
            # W and x loads split per kc-chunk so the first matmuls can
            # start as soon as their chunk lands.
            W_sb = singles.tile([128, 8, 3 * DL], bf16)
            Wl_r = Wl.rearrange("(kc p) n -> p kc n", p=128)
            for kc in range(8):
                nc.sync.dma_start(out=W_sb[:, kc, :], in_=Wl_r[:, kc, :])
            bqk_sb = singles.tile([128, 2 * NDQ], f32)
            nc.sync.dma_start(out=bqk_sb, in_=bqk)
            bv_row = singles.tile([1, DL], f32)
            nc.sync.dma_start(out=bv_row, in_=bv)
            bv_sb = singles.tile([128, DL], f32)
            nc.gpsimd.partition_broadcast(bv_sb, bv_row)

            ind = None
            if mode == "causal":
                ind = singles.tile([128, 4, 512], bf16)
                for j in range(4):
                    nc.vector.memset(ind[:, j, :], 1.0)
                    # keep (=1.0) iff f - p - 128*j >= 0, else 0.0
                    nc.gpsimd.affine_select(
                        out=ind[:, j, :],
                        in_=ind[:, j, :],
                        compare_op=mybir.AluOpType.is_ge,
                        fill=0.0,
                        base=-128 * j,
                        pattern=[[1, 512]],
                        channel_multiplier=-1,
                    )

            # ---- resident intermediates ----
            QT = singles.tile([128, NDQ, T], bf16)   # [dim%128, dimchunk, t]
            KT = singles.tile([128, NDQ, T], bf16)
            V1 = singles.tile([128, NKC, HL, D + 1], bf16)  # [t%128, kc, h, d+1]
            nc.vector.memset(V1[:, :, :, D : D + 1], 1.0)
            OT = singles.tile([128, NDQ, T], bf16)
            stage_sb = singles.tile([65, NQC, HL, 512], f32)
            Wp_sb = singles.tile([128, NDQ, C], bf16)
            nc.sync.dma_start(
                out=Wp_sb, in_=Wp.rearrange("(dq p) n -> p dq n", p=128)
            )

            # ---- phase 1: QKV projections ----
            for tb in range(NTB):
                x_sb = xin.tile([128, 8, 512], bf16)
                xr = xT.rearrange("(kc p) t -> p kc t", p=128)[
                    :, :, tb * 512 : (tb + 1) * 512
                ]
                for kc in range(8):
                    nc.sync.dma_start(out=x_sb[:, kc, :], in_=xr[:, kc, :])
                # Q^T and K^T, feature-major
                for s in range(2):  # 0=Q, 1=K
                    for dq in range(NDQ):
                        ps = ps512("qk")
                        col = s * DL + dq * 128
                        for kc in range(8):
                            nc.tensor.matmul(
                                ps,
                                lhsT=W_sb[:, kc, col : col + 128],
                                rhs=x_sb[:, kc, :],
                                start=(kc == 0),
                                stop=(kc == 7),
                            )
                        dst = (QT if s == 0 else KT)[
                            :, dq, tb * 512 : (tb + 1) * 512
                        ]
                        nc.vector.tensor_scalar_add(
                            dst, ps, bqk_sb[:, s * NDQ + dq : s * NDQ + dq + 1]
                        )
                # V natural [t, d] with bias, into V1
                for t4 in range(4):
                    tt = tb * 4 + t4
                    ps = ps512("v")
                    for kc in range(8):
                        nc.tensor.matmul(
                            ps[:, :DL],
                            lhsT=x_sb[:, kc, t4 * 128 : (t4 + 1) * 128],
                            rhs=W_sb[:, kc, 2 * DL : 3 * DL],
                            start=(kc == 0),
                            stop=(kc == 7),
                        )
                    nc.vector.tensor_tensor(
                        out=V1[:, tt, :, 0:D],
                        in0=ps[:, :DL].rearrange("p (h d) -> p h d", d=D),
                        in1=bv_sb.rearrange("p (h d) -> p h d", d=D),
                        op=add,
                    )

            if debug_dump:
                nc.sync.dma_start(out=dbg["qt_d"], in_=QT)
                nc.sync.dma_start(out=dbg["kt_d"], in_=KT)
                nc.sync.dma_start(out=dbg["v1_d"], in_=V1)

            # ---- phase 2: attention, S^T layout [k, q] ----
            # SBUF sources can't have partition-step 0; the per-qc softmax
            # denominators bounce through DRAM so the broadcast back into
            # OT's partition layout reads DRAM with a step-0 partition dim.
            rcp_dram = nc.dram_tensor(
                "rcp_scratch", [NQC, HL, 512], bf16, kind="Internal"
            ).ap()

            def head_pair(qc, hp, nkc, m_sb):
                ops = [ps512("o"), ps512("o")]  # per head in pair
                st_store = {}

                def emit_mm1(j):
                    stp = psum.tile(
                        [128, 2, 512], f32, name="ps1024", tag="ps1024", bufs=2
                    )
                    for hh in range(2):
                        off = 64 * hh
                        nc.tensor.matmul(
                            stp[:, hh, :],
                            lhsT=KT[off : off + 64, hp, j * 128 : (j + 1) * 128],
                            rhs=QT[
                                off : off + 64, hp, qc * 512 : (qc + 1) * 512
                            ],
                            start=True,
                            stop=True,
                        )
                    st_store[j] = stp

                emitted = min(2, nkc)  # lookahead 1 (PSUM-limited)
                for j in range(emitted):
                    emit_mm1(j)
                for kc in range(nkc):
                    stp = st_store.pop(kc)
                    p2 = ptiles.tile([128, 2, 512], bf16, tag="p")
                    if mode == "causal" and kc >= 4 * qc:
                        # exp only the columns the causal mask can reach;
                        # zero the fully-masked prefix, then apply the
                        # diagonal indicator to both heads at once.
                        j = kc - 4 * qc
                        w = 512 - 128 * j
                        if j > 0:
                            nc.vector.memset(p2[:, :, 0 : 128 * j], 0.0)
                        nc.scalar.activation(
                            p2[:, :, 128 * j :], stp[:, :, 128 * j :], Exp
                        )
                        base = ind[:, j, 128 * j :]
                        ind2 = bass.AP(
                            tensor=base.tensor,
                            offset=base.offset,
                            ap=[base.ap[0], [0, 2], base.ap[1]],
                        )
                        nc.vector.tensor_tensor(
                            out=p2[:, :, 128 * j :],
                            in0=p2[:, :, 128 * j :],
                            in1=ind2,
                            op=mult,
                        )
                    else:
                        nc.scalar.activation(p2, stp, Exp)
                        if mode == "general":
                            base = m_sb[:, kc, :]
                            msk2 = bass.AP(
                                tensor=base.tensor,
                                offset=base.offset,
                                ap=[base.ap[0], [0, 2], base.ap[1]],
                            )
                            nc.vector.tensor_tensor(
                                out=p2, in0=p2, in1=msk2, op=mult
                            )
                    for hh in range(2):
                        h = hp * 2 + hh
                        nc.tensor.matmul(
                            ops[hh][: D + 1, :],
                            lhsT=V1[:, kc, h, :],
                            rhs=p2[:, hh, :],
                            start=(kc == 0),
                            stop=(kc == nkc - 1),
                        )
                    if emitted < nkc:
                        emit_mm1(emitted)
                        emitted += 1
                # stash unnormalized output rows + softmax denominators
                # (denominator row stays on partition 64 -- engines can't
                # move data across partitions; DMA gathers it later)
                for hh in range(2):
                    h = hp * 2 + hh
                    off = 64 * hh
                    nc.vector.tensor_copy(
                        OT[off : off + 64, hp, qc * 512 : (qc + 1) * 512],
                        ops[hh][0:D, :],
                    )
                    nc.vector.tensor_copy(
                        stage_sb[64:65, qc, h, :], ops[hh][D : D + 1, :]
                    )

            for qc in range(NQC):
                nkc = 4 * qc + 4 if mode == "causal" else NKC
                m_sb = None
                if mode == "general":
                    m_sb = xin.tile([128, NKC, 512], bf16, tag="mask", bufs=1)
                    nc.sync.dma_start(
                        out=m_sb,
                        in_=maskT.rearrange("(kc p) q -> p kc q", p=128)[
                            :, :, qc * 512 : (qc + 1) * 512
                        ],
                    )
                for hp in range(NDQ):
                    head_pair(qc, hp, nkc, m_sb)

                # per-qc normalization (overlaps the next qc's attention)
                sums4 = small.tile([HL, 512], f32, tag="sums4", bufs=2)
                nc.gpsimd.dma_start(out=sums4, in_=stage_sb[64:65, qc, :, :])
                rcp4 = small.tile([HL, 512], f32, tag="rcp4", bufs=2)
                nc.vector.reciprocal(rcp4, sums4)
                rcpb4 = small.tile([HL, 512], bf16, tag="rcpb4", bufs=2)
                nc.vector.tensor_copy(rcpb4, rcp4)
                nc.sync.dma_start(out=rcp_dram[qc], in_=rcpb4)
                rb_qc = small.tile([128, NDQ, 512], bf16, tag="rb", bufs=2)
                for h in range(HL):
                    hp, off = h // 2, 64 * (h % 2)
                    src = rcp_dram[qc, h : h + 1, :]
                    src = bass.AP(
                        tensor=src.tensor,
                        offset=src.offset,
                        ap=[[0, 64], src.ap[-1]],
                    )
                    nc.gpsimd.dma_start(
                        out=rb_qc[off : off + 64, hp, :], in_=src
                    )
                nc.vector.tensor_tensor(
                    out=OT[:, :, qc * 512 : (qc + 1) * 512],
                    in0=OT[:, :, qc * 512 : (qc + 1) * 512],
                    in1=rb_qc,
                    op=mult,
                )

                # per-qc partial output projection
                for t4 in range(4):
                    tt = qc * 4 + t4
                    y_sb = outp.tile([128, C], f32)
                    for n in range(2):
                        pp = ps512("proj")
                        for dq in range(NDQ):
                            nc.tensor.matmul(
                                pp,
                                lhsT=OT[:, dq, tt * 128 : (tt + 1) * 128],
                                rhs=Wp_sb[:, dq, n * 512 : (n + 1) * 512],
                                start=(dq == 0),
                                stop=(dq == NDQ - 1),
                            )
                        nc.vector.tensor_copy(
                            y_sb[:, n * 512 : (n + 1) * 512], pp
                        )
                    nc.sync.dma_start(
                        out=yp[tt * 128 : (tt + 1) * 128, :], in_=y_sb
                    )

            if debug_dump:
                nc.sync.dma_start(out=dbg["ot_d"], in_=OT)

    nc.compile()
    return nc


def _host_prep(x, prefix_causal_mask, W_attn, b_attn, W_proj):
    """Split full inputs into 8 per-core input maps; detect mask mode."""
    scale = 1.0 / np.sqrt(np.float32(D))
    mask = np.asarray(prefix_causal_mask)
    if mask.all():
        mode = "full"
    else:
        tri = np.tril(np.ones((T, T), dtype=bool))
        if all(np.array_equal(mask[b], tri) for b in range(B)):
            mode = "causal"
        else:
            mode = "general"

    import ml_dtypes

    bf16 = ml_dtypes.bfloat16
    x = np.asarray(x, dtype=np.float32)
    W_attn = np.asarray(W_attn, dtype=np.float32)
    b_attn = np.asarray(b_attn, dtype=np.float32)
    W_proj = np.asarray(W_proj, dtype=np.float32)

    in_maps = []
    for core in range(NCORES):
        b = core // NHG
        hg = core % NHG
        lo = hg * DL
        hi = lo + DL
        xT = np.ascontiguousarray(x[b].T)  # [C, T]
        Wq = W_attn[:, lo:hi] * scale
        Wk = W_attn[:, C + lo : C + hi]
        Wv = W_attn[:, 2 * C + lo : 2 * C + hi]
        Wl = np.ascontiguousarray(np.concatenate([Wq, Wk, Wv], axis=1))
        bq = b_attn[lo:hi] * scale
        bk = b_attn[C + lo : C + hi]
        # bias per partition for Q,K chunks: cols = [q0, q1, k0, k1]
        bqk = np.stack(
            [bq[0:128], bq[128:256], bk[0:128], bk[128:256]], axis=1
        ).astype(np.float32)
        bv = np.ascontiguousarray(
            b_attn[2 * C + lo : 2 * C + hi][None, :]
        ).astype(np.float32)
        Wp = np.ascontiguousarray(W_proj[lo:hi, :])
        im = {
            "xT": xT.astype(bf16),
            "Wl": Wl.astype(bf16),
            "bqk": np.ascontiguousarray(bqk),
            "bv": bv,
            "Wp": Wp.astype(bf16),
        }
        if mode == "general":
            im["maskT"] = np.ascontiguousarray(mask[b].T).astype(bf16)
        in_maps.append(im)
    return mode, in_maps


def _get_program(mode):
    if mode not in _CACHE:
        _CACHE[mode] = _build(mode)
    return _CACHE[mode]


def _run(inputs, trace=False):
    """Returns (full_output [B,T,C], BassKernelResults)."""
    from concourse import bass_utils

    mode, in_maps = _host_prep(
        inputs["x"],
        inputs["prefix_causal_mask"],
        inputs["W_attn"],
        inputs["b_attn"],
        inputs["W_proj"],
    )
    nc = _get_program(mode)
    res = bass_utils.run_bass_kernel_spmd(
        nc, in_maps, core_ids=list(range(NCORES)), trace=trace
    )
    b_proj = np.asarray(inputs["b_proj"], dtype=np.float32)
    y = np.zeros((B, T, C), dtype=np.float32)
    for core in range(NCORES):
        y[core // NHG] += res.results[core]["yp"]
    y += b_proj[None, None, :]
    return y, res


def kernel(**inputs):
    y, _ = _run(inputs, trace=False)
    return y


# revision 31
# speedup vs baseline: 1.6206x; 1.2599x over previous
"""Causal self-attention for Trainium2, 8 NeuronCores.

Sharding: tensor-parallel over heads (4 heads/core) x data-parallel over
batch (2). Core i handles batch i//4, heads 4*(i%4)..4*(i%4)+3. Each core
computes its heads' attention output and a partial output projection
(W_proj rows for its heads); the host sums the 4 partials per batch and
adds b_proj.

Device layout choices:
  - Q^T, K^T computed feature-major [dim, t] directly (lhsT = W chunk,
    rhs = x^T chunk), so attention scores come out as S^T [k, q] with k
    on partitions -- which is exactly the layout the P@V matmul needs
    as its rhs. No on-chip transposes of the O(T^2) object.
  - V computed in natural [t, dim] layout (lhsT = x^T chunk, rhs = W_v),
    which is the lhsT layout the P@V matmul needs. A ones-column is
    appended to V so the softmax denominators fall out of the same
    matmul (row 64*... of the PSUM output).
  - exp() without max subtraction: scores are q.k/8 with q,k ~ N(0,1),
    bounded well inside fp32 exp range; softmax is shift-invariant so
    the result is mathematically identical to the reference.
  - all matmuls run as float32r (replicated fp32) with free dim >= 256,
    which streams at 1 column/cycle like bf16.

The causal mask is handled by skipping fully-masked k-chunks and
multiplying exp(S) by one of 4 precomputed 0/1 indicator tiles on the
diagonal-straddling chunks. If the runtime mask is not the lower-tri
causal mask, a general fallback multiplies by the actual mask (DMA'd
transposed) instead; an all-ones mask drops masking entirely.
"""

import numpy as np

B, T, C, H = 2, 2048, 1024, 16
D = C // H            # 64 head dim
NCORES = 8
NBG = 2               # batch shards
NHG = 4               # head-group shards
HL = H // NHG         # 4 heads per core
DL = HL * D           # 256 local feature dims
NDQ = DL // 128       # 2 partition chunks of local dims
NTB = T // 512        # 4 t-chunks of 512
NKC = T // 128        # 16 key chunks of 128
NQC = T // 512        # 4 query chunks of 512
NTT = T // 128        # 16 t-tiles of 128 (proj / V)

_CACHE = {}


def _build(mode, debug_dump=False):
    """Build + compile the per-core Bass program. mode: causal|full|general."""
    import concourse.bass as bass
    import concourse.bacc as bacc
    import concourse.tile as tile
    import concourse.mybir as mybir

    f32 = mybir.dt.float32
    bf16 = mybir.dt.bfloat16
    Exp = mybir.ActivationFunctionType.Exp
    Ident = mybir.ActivationFunctionType.Identity
    mult = mybir.AluOpType.mult
    add = mybir.AluOpType.add

    nc = bacc.Bacc(
        "TRN2", target_bir_lowering=False, debug=False, num_devices=NCORES
    )

    xT = nc.dram_tensor("xT", [C, T], bf16, kind="ExternalInput").ap()
    Wl = nc.dram_tensor("Wl", [C, 3 * DL], bf16, kind="ExternalInput").ap()
    bqk = nc.dram_tensor("bqk", [128, 2 * NDQ], f32, kind="ExternalInput").ap()
    bv = nc.dram_tensor("bv", [1, DL], f32, kind="ExternalInput").ap()
    Wp = nc.dram_tensor("Wp", [DL, C], bf16, kind="ExternalInput").ap()
    maskT = None
    if mode == "general":
        maskT = nc.dram_tensor("maskT", [T, T], bf16, kind="ExternalInput").ap()
    yp = nc.dram_tensor("yp", [T, C], f32, kind="ExternalOutput").ap()
    dbg = {}
    if debug_dump:
        for nm, shp, dt in [
            ("qt_d", [128, NDQ, T], bf16), ("kt_d", [128, NDQ, T], bf16),
            ("v1_d", [128, NKC, HL, D + 1], bf16), ("ot_d", [128, NDQ, T], bf16),
            ("st_d", [128, 512], f32), ("p_d", [128, 512], bf16),
            ("o_d", [65, 512], f32),
        ]:
            dbg[nm] = nc.dram_tensor(nm, shp, dt, kind="ExternalOutput").ap()

    with tile.TileContext(nc) as tc:
        with (
            tc.tile_pool(name="singles", bufs=1) as singles,
            tc.tile_pool(name="xin", bufs=2) as xin,
            tc.tile_pool(name="ptiles", bufs=6) as ptiles,
            tc.tile_pool(name="small", bufs=4) as small,
            tc.tile_pool(name="outp", bufs=3) as outp,
            tc.tile_pool(name="psum", bufs=7, space="PSUM") as psum,
        ):
            def ps512(name):
                return psum.tile(
                    [128, 512], f32, name="ps512", tag="ps512", bufs=4
                )

            # ---- resident inputs ----
            # W and x loads split per kc-chunk so the first matmuls can
            # start as soon as their chunk lands.
            W_sb = singles.tile([128, 8, 3 * DL], bf16)
            Wl_r = Wl.rearrange("(kc p) n -> p kc n", p=128)
            x0_sb = xin.tile([128, 8, 512], bf16, tag="x_sb", name="x_sb")
            x0r = xT.rearrange("(kc p) t -> p kc t", p=128)[:, :, 0:512]
            for kc in range(8):
                nc.sync.dma_start(out=W_sb[:, kc, :], in_=Wl_r[:, kc, :])
                nc.sync.dma_start(out=x0_sb[:, kc, :], in_=x0r[:, kc, :])
            bqk_sb = singles.tile([128, 2 * NDQ], f32)
            nc.sync.dma_start(out=bqk_sb, in_=bqk)
            bv_row = singles.tile([1, DL], f32)
            nc.sync.dma_start(out=bv_row, in_=bv)
            bv_sb = singles.tile([128, DL], f32)
            nc.gpsimd.partition_broadcast(bv_sb, bv_row)

            ind = None
            if mode == "causal":
                ind = singles.tile([128, 4, 512], bf16)
                for j in range(4):
                    nc.vector.memset(ind[:, j, :], 1.0)
                    # keep (=1.0) iff f - p - 128*j >= 0, else 0.0
                    nc.gpsimd.affine_select(
                        out=ind[:, j, :],
                        in_=ind[:, j, :],
                        compare_op=mybir.AluOpType.is_ge,
                        fill=0.0,
                        base=-128 * j,
                        pattern=[[1, 512]],
                        channel_multiplier=-1,
                    )

            # ---- resident intermediates ----
            QT = singles.tile([128, NDQ, T], bf16)   # [dim%128, dimchunk, t]
            KT = singles.tile([128, NDQ, T], bf16)
            V1 = singles.tile([128, NKC, HL, D + 1], bf16)  # [t%128, kc, h, d+1]
            nc.vector.memset(V1[:, :, :, D : D + 1], 1.0)
            OT = singles.tile([128, NDQ, T], bf16)
            stage_sb = singles.tile([65, NQC, HL, 512], f32)
            Wp_sb = singles.tile([128, NDQ, C], bf16)

            # ---- phase 1: QKV projections ----
            for tb in range(NTB):
                if tb == 0:
                    x_sb = x0_sb
                else:
                    x_sb = xin.tile([128, 8, 512], bf16, tag="x_sb", name="x_sb")
                    xr = xT.rearrange("(kc p) t -> p kc t", p=128)[
                        :, :, tb * 512 : (tb + 1) * 512
                    ]
                    for kc in range(8):
                        nc.sync.dma_start(out=x_sb[:, kc, :], in_=xr[:, kc, :])
                # Q^T and K^T, feature-major
                for s in range(2):  # 0=Q, 1=K
                    for dq in range(NDQ):
                        ps = ps512("qk")
                        col = s * DL + dq * 128
                        for kc in range(8):
                            nc.tensor.matmul(
                                ps,
                                lhsT=W_sb[:, kc, col : col + 128],
                                rhs=x_sb[:, kc, :],
                                start=(kc == 0),
                                stop=(kc == 7),
                            )
                        dst = (QT if s == 0 else KT)[
                            :, dq, tb * 512 : (tb + 1) * 512
                        ]
                        nc.vector.tensor_scalar_add(
                            dst, ps, bqk_sb[:, s * NDQ + dq : s * NDQ + dq + 1]
                        )
                # V natural [t, d] with bias, into V1
                for t4 in range(4):
                    tt = tb * 4 + t4
                    ps = ps512("v")
                    for kc in range(8):
                        nc.tensor.matmul(
                            ps[:, :DL],
                            lhsT=x_sb[:, kc, t4 * 128 : (t4 + 1) * 128],
                            rhs=W_sb[:, kc, 2 * DL : 3 * DL],
                            start=(kc == 0),
                            stop=(kc == 7),
                        )
                    nc.vector.tensor_tensor(
                        out=V1[:, tt, :, 0:D],
                        in0=ps[:, :DL].rearrange("p (h d) -> p h d", d=D),
                        in1=bv_sb.rearrange("p (h d) -> p h d", d=D),
                        op=add,
                    )

            nc.sync.dma_start(
                out=Wp_sb, in_=Wp.rearrange("(dq p) n -> p dq n", p=128)
            )

            if debug_dump:
                nc.sync.dma_start(out=dbg["qt_d"], in_=QT)
                nc.sync.dma_start(out=dbg["kt_d"], in_=KT)
                nc.sync.dma_start(out=dbg["v1_d"], in_=V1)

            # ---- phase 2: attention, S^T layout [k, q] ----
            # SBUF sources can't have partition-step 0; the per-qc softmax
            # denominators bounce through DRAM so the broadcast back into
            # OT's partition layout reads DRAM with a step-0 partition dim.
            rcp_dram = nc.dram_tensor(
                "rcp_scratch", [NQC, HL, 512], bf16, kind="Internal"
            ).ap()

            def proj_qc(qc):
                for t4 in range(4):
                    tt = qc * 4 + t4
                    y_sb = outp.tile([128, C], f32, name="y_sb")
                    for n in range(2):
                        pp = ps512("proj")
                        for dq in range(NDQ):
                            nc.tensor.matmul(
                                pp,
                                lhsT=OT[:, dq, tt * 128 : (tt + 1) * 128],
                                rhs=Wp_sb[:, dq, n * 512 : (n + 1) * 512],
                                start=(dq == 0),
                                stop=(dq == NDQ - 1),
                            )
                        nc.vector.tensor_copy(
                            y_sb[:, n * 512 : (n + 1) * 512], pp
                        )
                    nc.sync.dma_start(
                        out=yp[tt * 128 : (tt + 1) * 128, :], in_=y_sb
                    )

            def head_pair(qc, hp, nkc, m_sb):
                ops = [ps512("o"), ps512("o")]  # per head in pair
                st_store = {}

                def emit_mm1(j):
                    stp = psum.tile(
                        [128, 2, 512], f32, name="ps1024", tag="ps1024", bufs=2
                    )
                    for hh in range(2):
                        off = 64 * hh
                        nc.tensor.matmul(
                            stp[:, hh, :],
                            lhsT=KT[off : off + 64, hp, j * 128 : (j + 1) * 128],
                            rhs=QT[
                                off : off + 64, hp, qc * 512 : (qc + 1) * 512
                            ],
                            start=True,
                            stop=True,
                        )
                    st_store[j] = stp

                emitted = min(2, nkc)  # lookahead 1 (PSUM-limited)
                for j in range(emitted):
                    emit_mm1(j)
                for kc in range(nkc):
                    stp = st_store.pop(kc)
                    p2 = ptiles.tile([128, 2, 512], bf16, tag="p")
                    if mode == "causal" and kc >= 4 * qc:
                        # exp only the columns the causal mask can reach;
                        # zero the fully-masked prefix, then apply the
                        # diagonal indicator to both heads at once.
                        j = kc - 4 * qc
                        w = 512 - 128 * j
                        if j > 0:
                            nc.vector.memset(p2[:, :, 0 : 128 * j], 0.0)
                        nc.scalar.activation(
                            p2[:, :, 128 * j :], stp[:, :, 128 * j :], Exp
                        )
                        base = ind[:, j, 128 * j :]
                        ind2 = bass.AP(
                            tensor=base.tensor,
                            offset=base.offset,
                            ap=[base.ap[0], [0, 2], base.ap[1]],
                        )
                        nc.vector.tensor_tensor(
                            out=p2[:, :, 128 * j :],
                            in0=p2[:, :, 128 * j :],
                            in1=ind2,
                            op=mult,
                        )
                    else:
                        nc.scalar.activation(p2, stp, Exp)
                        if mode == "general":
                            base = m_sb[:, kc, :]
                            msk2 = bass.AP(
                                tensor=base.tensor,
                                offset=base.offset,
                                ap=[base.ap[0], [0, 2], base.ap[1]],
                            )
                            nc.vector.tensor_tensor(
                                out=p2, in0=p2, in1=msk2, op=mult
                            )
                    for hh in range(2):
                        h = hp * 2 + hh
                        nc.tensor.matmul(
                            ops[hh][: D + 1, :],
                            lhsT=V1[:, kc, h, :],
                            rhs=p2[:, hh, :],
                            start=(kc == 0),
                            stop=(kc == nkc - 1),
                        )
                    if emitted < nkc:
                        emit_mm1(emitted)
                        emitted += 1
                # stash unnormalized output rows + softmax denominators
                # (denominator row stays on partition 64 -- engines can't
                # move data across partitions; DMA gathers it later)
                for hh in range(2):
                    h = hp * 2 + hh
                    off = 64 * hh
                    nc.vector.tensor_copy(
                        OT[off : off + 64, hp, qc * 512 : (qc + 1) * 512],
                        ops[hh][0:D, :],
                    )
                    nc.vector.tensor_copy(
                        stage_sb[64:65, qc, h, :], ops[hh][D : D + 1, :]
                    )

            for qc in range(NQC):
                nkc = 4 * qc + 4 if mode == "causal" else NKC
                m_sb = None
                if mode == "general":
                    m_sb = xin.tile([128, NKC, 512], bf16, tag="mask", bufs=1)
                    nc.sync.dma_start(
                        out=m_sb,
                        in_=maskT.rearrange("(kc p) q -> p kc q", p=128)[
                            :, :, qc * 512 : (qc + 1) * 512
                        ],
                    )
                for hp in range(NDQ):
                    head_pair(qc, hp, nkc, m_sb)

                # per-qc normalization (overlaps the next qc's attention)
                sums4 = small.tile([HL, 512], f32, tag="sums4", bufs=2)
                nc.gpsimd.dma_start(out=sums4, in_=stage_sb[64:65, qc, :, :])
                rcp4 = small.tile([HL, 512], f32, tag="rcp4", bufs=2)
                nc.vector.reciprocal(rcp4, sums4)
                rcpb4 = small.tile([HL, 512], bf16, tag="rcpb4", bufs=2)
                nc.vector.tensor_copy(rcpb4, rcp4)
                nc.sync.dma_start(out=rcp_dram[qc], in_=rcpb4)
                rb_qc = small.tile([128, NDQ, 512], bf16, tag="rb", bufs=2)
                for h in range(HL):
                    hp, off = h // 2, 64 * (h % 2)
                    src = rcp_dram[qc, h : h + 1, :]
                    src = bass.AP(
                        tensor=src.tensor,
                        offset=src.offset,
                        ap=[[0, 64], src.ap[-1]],
                    )
                    nc.gpsimd.dma_start(
                        out=rb_qc[off : off + 64, hp, :], in_=src
                    )
                nc.vector.tensor_tensor(
                    out=OT[:, :, qc * 512 : (qc + 1) * 512],
                    in0=OT[:, :, qc * 512 : (qc + 1) * 512],
                    in1=rb_qc,
                    op=mult,
                )

                # projection runs one qc behind so its matmuls never wait
                # on the normalize chain in the PE FIFO
                if qc >= 1:
                    proj_qc(qc - 1)
            proj_qc(NQC - 1)

            if debug_dump:
                nc.sync.dma_start(out=dbg["ot_d"], in_=OT)

    nc.compile()
    return nc


def _host_prep(x, prefix_causal_mask, W_attn, b_attn, W_proj):
    """Split full inputs into 8 per-core input maps; detect mask mode."""
    scale = 1.0 / np.sqrt(np.float32(D))
    mask = np.asarray(prefix_causal_mask)
    if mask.all():
        mode = "full"
    else:
        tri = np.tril(np.ones((T, T), dtype=bool))
        if all(np.array_equal(mask[b], tri) for b in range(B)):
            mode = "causal"
        else:
            mode = "general"

    import ml_dtypes

    bf16 = ml_dtypes.bfloat16
    x = np.asarray(x, dtype=np.float32)
    W_attn = np.asarray(W_attn, dtype=np.float32)
    b_attn = np.asarray(b_attn, dtype=np.float32)
    W_proj = np.asarray(W_proj, dtype=np.float32)

    in_maps = []
    for core in range(NCORES):
        b = core // NHG
        hg = core % NHG
        lo = hg * DL
        hi = lo + DL
        xT = np.ascontiguousarray(x[b].T)  # [C, T]
        Wq = W_attn[:, lo:hi] * scale
        Wk = W_attn[:, C + lo : C + hi]
        Wv = W_attn[:, 2 * C + lo : 2 * C + hi]
        Wl = np.ascontiguousarray(np.concatenate([Wq, Wk, Wv], axis=1))
        bq = b_attn[lo:hi] * scale
        bk = b_attn[C + lo : C + hi]
        # bias per partition for Q,K chunks: cols = [q0, q1, k0, k1]
        bqk = np.stack(
            [bq[0:128], bq[128:256], bk[0:128], bk[128:256]], axis=1
        ).astype(np.float32)
        bv = np.ascontiguousarray(
            b_attn[2 * C + lo : 2 * C + hi][None, :]
        ).astype(np.float32)
        Wp = np.ascontiguousarray(W_proj[lo:hi, :])
        im = {
            "xT": xT.astype(bf16),
            "Wl": Wl.astype(bf16),
            "bqk": np.ascontiguousarray(bqk),
            "bv": bv,
            "Wp": Wp.astype(bf16),
        }
        if mode == "general":
            im["maskT"] = np.ascontiguousarray(mask[b].T).astype(bf16)
        in_maps.append(im)
    return mode, in_maps


def _get_program(mode):
    if mode not in _CACHE:
        _CACHE[mode] = _build(mode)
    return _CACHE[mode]


def _run(inputs, trace=False):
    """Returns (full_output [B,T,C], BassKernelResults)."""
    from concourse import bass_utils

    mode, in_maps = _host_prep(
        inputs["x"],
        inputs["prefix_causal_mask"],
        inputs["W_attn"],
        inputs["b_attn"],
        inputs["W_proj"],
    )
    nc = _get_program(mode)
    res = bass_utils.run_bass_kernel_spmd(
        nc, in_maps, core_ids=list(range(NCORES)), trace=trace
    )
    b_proj = np.asarray(inputs["b_proj"], dtype=np.float32)
    y = np.zeros((B, T, C), dtype=np.float32)
    for core in range(NCORES):
        y[core // NHG] += res.results[core]["yp"]
    y += b_proj[None, None, :]
    return y, res


def kernel(**inputs):
    y, _ = _run(inputs, trace=False)
    return y


# revision 33
# speedup vs baseline: 1.6329x; 1.0076x over previous
"""Causal self-attention for Trainium2, 8 NeuronCores.

Sharding: tensor-parallel over heads (4 heads/core) x data-parallel over
batch (2). Core i handles batch i//4, heads 4*(i%4)..4*(i%4)+3. Each core
computes its heads' attention output and a partial output projection
(W_proj rows for its heads); the host sums the 4 partials per batch and
adds b_proj.

Device layout choices:
  - Q^T, K^T computed feature-major [dim, t] directly (lhsT = W chunk,
    rhs = x^T chunk), so attention scores come out as S^T [k, q] with k
    on partitions -- which is exactly the layout the P@V matmul needs
    as its rhs. No on-chip transposes of the O(T^2) object.
  - V computed in natural [t, dim] layout (lhsT = x^T chunk, rhs = W_v),
    which is the lhsT layout the P@V matmul needs. A ones-column is
    appended to V so the softmax denominators fall out of the same
    matmul (row 64*... of the PSUM output).
  - exp() without max subtraction: scores are q.k/8 with q,k ~ N(0,1),
    bounded well inside fp32 exp range; softmax is shift-invariant so
    the result is mathematically identical to the reference.
  - all matmuls run as float32r (replicated fp32) with free dim >= 256,
    which streams at 1 column/cycle like bf16.

The causal mask is handled by skipping fully-masked k-chunks and
multiplying exp(S) by one of 4 precomputed 0/1 indicator tiles on the
diagonal-straddling chunks. If the runtime mask is not the lower-tri
causal mask, a general fallback multiplies by the actual mask (DMA'd
transposed) instead; an all-ones mask drops masking entirely.
"""

import numpy as np

B, T, C, H = 2, 2048, 1024, 16
D = C // H            # 64 head dim
NCORES = 8
NBG = 2               # batch shards
NHG = 4               # head-group shards
HL = H // NHG         # 4 heads per core
DL = HL * D           # 256 local feature dims
NDQ = DL // 128       # 2 partition chunks of local dims
NTB = T // 512        # 4 t-chunks of 512
NKC = T // 128        # 16 key chunks of 128
NQC = T // 512        # 4 query chunks of 512
NTT = T // 128        # 16 t-tiles of 128 (proj / V)

_CACHE = {}


def _build(mode, debug_dump=False):
    """Build + compile the per-core Bass program. mode: causal|full|general."""
    import concourse.bass as bass
    import concourse.bacc as bacc
    import concourse.tile as tile
    import concourse.mybir as mybir

    f32 = mybir.dt.float32
    bf16 = mybir.dt.bfloat16
    Exp = mybir.ActivationFunctionType.Exp
    Ident = mybir.ActivationFunctionType.Identity
    mult = mybir.AluOpType.mult
    add = mybir.AluOpType.add

    nc = bacc.Bacc(
        "TRN2", target_bir_lowering=False, debug=False, num_devices=NCORES
    )

    xT = nc.dram_tensor("xT", [C, T], bf16, kind="ExternalInput").ap()
    Wl = nc.dram_tensor("Wl", [C, 3 * DL], bf16, kind="ExternalInput").ap()
    bqk = nc.dram_tensor("bqk", [128, 2 * NDQ], f32, kind="ExternalInput").ap()
    bv = nc.dram_tensor("bv", [1, DL], f32, kind="ExternalInput").ap()
    Wp = nc.dram_tensor("Wp", [DL, C], bf16, kind="ExternalInput").ap()
    maskT = None
    if mode == "general":
        maskT = nc.dram_tensor("maskT", [T, T], bf16, kind="ExternalInput").ap()
    yp = nc.dram_tensor("yp", [T, C], f32, kind="ExternalOutput").ap()
    dbg = {}
    if debug_dump:
        for nm, shp, dt in [
            ("qt_d", [128, NDQ, T], bf16), ("kt_d", [128, NDQ, T], bf16),
            ("v1_d", [128, NKC, HL, D + 1], bf16), ("ot_d", [128, NDQ, T], bf16),
            ("st_d", [128, 512], f32), ("p_d", [128, 512], bf16),
            ("o_d", [65, 512], f32),
        ]:
            dbg[nm] = nc.dram_tensor(nm, shp, dt, kind="ExternalOutput").ap()

    with tile.TileContext(nc) as tc:
        with (
            tc.tile_pool(name="singles", bufs=1) as singles,
            tc.tile_pool(name="xin", bufs=2) as xin,
            tc.tile_pool(name="ptiles", bufs=6) as ptiles,
            tc.tile_pool(name="small", bufs=4) as small,
            tc.tile_pool(name="outp", bufs=3) as outp,
            tc.tile_pool(name="psum", bufs=7, space="PSUM") as psum,
        ):
            def ps512(name):
                return psum.tile(
                    [128, 512], f32, name="ps512", tag="ps512", bufs=4
                )

            # ---- resident inputs ----
            # W and x loads split per kc-chunk so the first matmuls can
            # start as soon as their chunk lands.
            W_sb = singles.tile([128, 8, 3 * DL], bf16)
            Wl_r = Wl.rearrange("(kc p) n -> p kc n", p=128)
            x0_sb = xin.tile([128, 8, 512], bf16, tag="x_sb", name="x_sb")
            x0r = xT.rearrange("(kc p) t -> p kc t", p=128)[:, :, 0:512]
            for kc in range(8):
                nc.sync.dma_start(out=W_sb[:, kc, :], in_=Wl_r[:, kc, :])
                nc.sync.dma_start(out=x0_sb[:, kc, :], in_=x0r[:, kc, :])
            bqk_sb = singles.tile([128, 2 * NDQ], f32)
            nc.sync.dma_start(out=bqk_sb, in_=bqk)
            bv_row = singles.tile([1, DL], f32)
            nc.sync.dma_start(out=bv_row, in_=bv)
            bv_sb = singles.tile([128, DL], f32)
            nc.gpsimd.partition_broadcast(bv_sb, bv_row)

            ind = None
            if mode == "causal":
                ind = singles.tile([128, 4, 512], bf16)
                for j in range(4):
                    nc.vector.memset(ind[:, j, :], 1.0)
                    # keep (=1.0) iff f - p - 128*j >= 0, else 0.0
                    nc.gpsimd.affine_select(
                        out=ind[:, j, :],
                        in_=ind[:, j, :],
                        compare_op=mybir.AluOpType.is_ge,
                        fill=0.0,
                        base=-128 * j,
                        pattern=[[1, 512]],
                        channel_multiplier=-1,
                    )

            # ---- resident intermediates ----
            QT = singles.tile([128, NDQ, T], bf16)   # [dim%128, dimchunk, t]
            KT = singles.tile([128, NDQ, T], bf16)
            V1 = singles.tile([128, NKC, HL, D + 1], bf16)  # [t%128, kc, h, d+1]
            nc.vector.memset(V1[:, :, :, D : D + 1], 1.0)
            OT = singles.tile([128, NDQ, T], bf16)
            stage_sb = singles.tile([65, NQC, HL, 512], f32)
            Wp_sb = singles.tile([128, NDQ, C], bf16)

            # ---- phase 1: QKV projections (as interleavable units) ----
            def p1_units(tb, x_sb):
                """Units for one 512-wide t-chunk of the QKV projection."""
                units = []
                if tb > 0:
                    def dma_u(tb=tb, x_sb=x_sb):
                        xr = xT.rearrange("(kc p) t -> p kc t", p=128)[
                            :, :, tb * 512 : (tb + 1) * 512
                        ]
                        for kc in range(8):
                            nc.sync.dma_start(
                                out=x_sb[:, kc, :], in_=xr[:, kc, :]
                            )
                    units.append(dma_u)
                for s in range(2):  # 0=Q, 1=K
                    for dq in range(NDQ):
                        def qk_u(tb=tb, s=s, dq=dq, x_sb=x_sb):
                            ps = ps512("qk")
                            col = s * DL + dq * 128
                            for kc in range(8):
                                nc.tensor.matmul(
                                    ps,
                                    lhsT=W_sb[:, kc, col : col + 128],
                                    rhs=x_sb[:, kc, :],
                                    start=(kc == 0),
                                    stop=(kc == 7),
                                )
                            dst = (QT if s == 0 else KT)[
                                :, dq, tb * 512 : (tb + 1) * 512
                            ]
                            nc.vector.tensor_scalar_add(
                                dst, ps,
                                bqk_sb[:, s * NDQ + dq : s * NDQ + dq + 1],
                            )
                        units.append(qk_u)
                for t4 in range(4):
                    def v_u(tb=tb, t4=t4, x_sb=x_sb):
                        tt = tb * 4 + t4
                        ps = ps512("v")
                        for kc in range(8):
                            nc.tensor.matmul(
                                ps[:, :DL],
                                lhsT=x_sb[:, kc, t4 * 128 : (t4 + 1) * 128],
                                rhs=W_sb[:, kc, 2 * DL : 3 * DL],
                                start=(kc == 0),
                                stop=(kc == 7),
                            )
                        nc.vector.tensor_tensor(
                            out=V1[:, tt, :, 0:D],
                            in0=ps[:, :DL].rearrange("p (h d) -> p h d", d=D),
                            in1=bv_sb.rearrange("p (h d) -> p h d", d=D),
                            op=add,
                        )
                    units.append(v_u)
                return units

            # ---- phase 2/3 units ----
            # SBUF sources can't have partition-step 0; the per-qc softmax
            # denominators bounce through DRAM so the broadcast back into
            # OT's partition layout reads DRAM with a step-0 partition dim.
            rcp_dram = nc.dram_tensor(
                "rcp_scratch", [NQC, HL, 512], bf16, kind="Internal"
            ).ap()

            def proj_units(qc):
                units = []
                for t4 in range(4):
                    def u(qc=qc, t4=t4):
                        tt = qc * 4 + t4
                        y_sb = outp.tile([128, C], f32, name="y_sb")
                        for n in range(2):
                            pp = ps512("proj")
                            for dq in range(NDQ):
                                nc.tensor.matmul(
                                    pp,
                                    lhsT=OT[:, dq, tt * 128 : (tt + 1) * 128],
                                    rhs=Wp_sb[:, dq, n * 512 : (n + 1) * 512],
                                    start=(dq == 0),
                                    stop=(dq == NDQ - 1),
                                )
                            nc.vector.tensor_copy(
                                y_sb[:, n * 512 : (n + 1) * 512], pp
                            )
                        nc.sync.dma_start(
                            out=yp[tt * 128 : (tt + 1) * 128, :], in_=y_sb
                        )
                    units.append(u)
                return units

            def attn_units(qc, hp, nkc, m_sb):
                """One head-pair's attention over all k-chunks, softmax
                denominators via the ones-column of V1."""
                state = {}

                def emit_mm1(j):
                    stp = psum.tile(
                        [128, 2, 512], f32, name="ps1024", tag="ps1024", bufs=2
                    )
                    for hh in range(2):
                        off = 64 * hh
                        nc.tensor.matmul(
                            stp[:, hh, :],
                            lhsT=KT[off : off + 64, hp, j * 128 : (j + 1) * 128],
                            rhs=QT[
                                off : off + 64, hp, qc * 512 : (qc + 1) * 512
                            ],
                            start=True,
                            stop=True,
                        )
                    state.setdefault("st", {})[j] = stp

                def prologue():
                    state["ops"] = [ps512("o"), ps512("o")]
                    state["emitted"] = min(2, nkc)  # lookahead 1
                    for j in range(state["emitted"]):
                        emit_mm1(j)

                def consume(kc):
                    ops = state["ops"]
                    stp = state["st"].pop(kc)
                    p2 = ptiles.tile([128, 2, 512], bf16, tag="p")
                    if mode == "causal" and kc >= 4 * qc:
                        # exp only the columns the causal mask can reach;
                        # zero the fully-masked prefix, then apply the
                        # diagonal indicator to both heads at once.
                        j = kc - 4 * qc
                        if j > 0:
                            nc.vector.memset(p2[:, :, 0 : 128 * j], 0.0)
                        nc.scalar.activation(
                            p2[:, :, 128 * j :], stp[:, :, 128 * j :], Exp
                        )
                        base = ind[:, j, 128 * j :]
                        ind2 = bass.AP(
                            tensor=base.tensor,
                            offset=base.offset,
                            ap=[base.ap[0], [0, 2], base.ap[1]],
                        )
                        nc.vector.tensor_tensor(
                            out=p2[:, :, 128 * j :],
                            in0=p2[:, :, 128 * j :],
                            in1=ind2,
                            op=mult,
                        )
                    else:
                        nc.scalar.activation(p2, stp, Exp)
                        if mode == "general":
                            base = m_sb[:, kc, :]
                            msk2 = bass.AP(
                                tensor=base.tensor,
                                offset=base.offset,
                                ap=[base.ap[0], [0, 2], base.ap[1]],
                            )
                            nc.vector.tensor_tensor(
                                out=p2, in0=p2, in1=msk2, op=mult
                            )
                    for hh in range(2):
                        h = hp * 2 + hh
                        nc.tensor.matmul(
                            ops[hh][: D + 1, :],
                            lhsT=V1[:, kc, h, :],
                            rhs=p2[:, hh, :],
                            start=(kc == 0),
                            stop=(kc == nkc - 1),
                        )
                    if state["emitted"] < nkc:
                        emit_mm1(state["emitted"])
                        state["emitted"] += 1

                def stash():
                    # unnormalized output rows + denominators (the
                    # denominator row stays on partition 64 -- engines
                    # can't move data across partitions; DMA gathers it)
                    ops = state["ops"]
                    for hh in range(2):
                        h = hp * 2 + hh
                        off = 64 * hh
                        nc.vector.tensor_copy(
                            OT[off : off + 64, hp, qc * 512 : (qc + 1) * 512],
                            ops[hh][0:D, :],
                        )
                        nc.vector.tensor_copy(
                            stage_sb[64:65, qc, h, :], ops[hh][D : D + 1, :]
                        )

                units = [prologue]
                for kc in range(nkc):
                    units.append(lambda kc=kc: consume(kc))
                units.append(stash)
                return units

            def norm_qc(qc):
                # per-qc normalization (overlaps the next qc's attention)
                sums4 = small.tile([HL, 512], f32, tag="sums4", bufs=2)
                nc.gpsimd.dma_start(out=sums4, in_=stage_sb[64:65, qc, :, :])
                rcp4 = small.tile([HL, 512], f32, tag="rcp4", bufs=2)
                nc.vector.reciprocal(rcp4, sums4)
                rcpb4 = small.tile([HL, 512], bf16, tag="rcpb4", bufs=2)
                nc.vector.tensor_copy(rcpb4, rcp4)
                nc.sync.dma_start(out=rcp_dram[qc], in_=rcpb4)
                rb_qc = small.tile([128, NDQ, 512], bf16, tag="rb", bufs=2)
                for h in range(HL):
                    hp, off = h // 2, 64 * (h % 2)
                    src = rcp_dram[qc, h : h + 1, :]
                    src = bass.AP(
                        tensor=src.tensor,
                        offset=src.offset,
                        ap=[[0, 64], src.ap[-1]],
                    )
                    nc.gpsimd.dma_start(
                        out=rb_qc[off : off + 64, hp, :], in_=src
                    )
                nc.vector.tensor_tensor(
                    out=OT[:, :, qc * 512 : (qc + 1) * 512],
                    in0=OT[:, :, qc * 512 : (qc + 1) * 512],
                    in1=rb_qc,
                    op=mult,
                )

            # ---- schedule: staircase interleave ----
            # attn(qc) needs phase-1 chunks tb <= qc only, so phase-1(tb+1)
            # and proj(qc-1) units are injected between attention units to
            # keep the PE FIFO fed while ACT paces the exp chain.
            for u in p1_units(0, x0_sb):
                u()
            nc.sync.dma_start(
                out=Wp_sb, in_=Wp.rearrange("(dq p) n -> p dq n", p=128)
            )
            for qc in range(NQC):
                nkc = 4 * qc + 4 if mode == "causal" else NKC
                m_sb = None
                if mode == "general":
                    m_sb = xin.tile([128, NKC, 512], bf16, tag="mask", bufs=1)
                    nc.sync.dma_start(
                        out=m_sb,
                        in_=maskT.rearrange("(kc p) q -> p kc q", p=128)[
                            :, :, qc * 512 : (qc + 1) * 512
                        ],
                    )
                inj = []
                if qc + 1 < NTB:
                    x_next = xin.tile(
                        [128, 8, 512], bf16, tag="x_sb", name="x_sb"
                    )
                    inj += p1_units(qc + 1, x_next)
                if qc >= 1:
                    inj += proj_units(qc - 1)
                main = []
                for hp in range(NDQ):
                    main += attn_units(qc, hp, nkc, m_sb)
                k, m, j = len(main), len(inj), 0
                for i, u in enumerate(main):
                    u()
                    take = (i + 1) * m // k - i * m // k
                    for _ in range(take):
                        inj[j]()
                        j += 1
                norm_qc(qc)
            for u in proj_units(NQC - 1):
                u()

            if debug_dump:
                nc.sync.dma_start(out=dbg["ot_d"], in_=OT)

    nc.compile()
    return nc


def _host_prep(x, prefix_causal_mask, W_attn, b_attn, W_proj):
    """Split full inputs into 8 per-core input maps; detect mask mode."""
    scale = 1.0 / np.sqrt(np.float32(D))
    mask = np.asarray(prefix_causal_mask)
    if mask.all():
        mode = "full"
    else:
        tri = np.tril(np.ones((T, T), dtype=bool))
        if all(np.array_equal(mask[b], tri) for b in range(B)):
            mode = "causal"
        else:
            mode = "general"

    import ml_dtypes

    bf16 = ml_dtypes.bfloat16
    x = np.asarray(x, dtype=np.float32)
    W_attn = np.asarray(W_attn, dtype=np.float32)
    b_attn = np.asarray(b_attn, dtype=np.float32)
    W_proj = np.asarray(W_proj, dtype=np.float32)

    in_maps = []
    for core in range(NCORES):
        b = core // NHG
        hg = core % NHG
        lo = hg * DL
        hi = lo + DL
        xT = np.ascontiguousarray(x[b].T)  # [C, T]
        Wq = W_attn[:, lo:hi] * scale
        Wk = W_attn[:, C + lo : C + hi]
        Wv = W_attn[:, 2 * C + lo : 2 * C + hi]
        Wl = np.ascontiguousarray(np.concatenate([Wq, Wk, Wv], axis=1))
        bq = b_attn[lo:hi] * scale
        bk = b_attn[C + lo : C + hi]
        # bias per partition for Q,K chunks: cols = [q0, q1, k0, k1]
        bqk = np.stack(
            [bq[0:128], bq[128:256], bk[0:128], bk[128:256]], axis=1
        ).astype(np.float32)
        bv = np.ascontiguousarray(
            b_attn[2 * C + lo : 2 * C + hi][None, :]
        ).astype(np.float32)
        Wp = np.ascontiguousarray(W_proj[lo:hi, :])
        im = {
            "xT": xT.astype(bf16),
            "Wl": Wl.astype(bf16),
            "bqk": np.ascontiguousarray(bqk),
            "bv": bv,
            "Wp": Wp.astype(bf16),
        }
        if mode == "general":
            im["maskT"] = np.ascontiguousarray(mask[b].T).astype(bf16)
        in_maps.append(im)
    return mode, in_maps


def _get_program(mode):
    if mode not in _CACHE:
        _CACHE[mode] = _build(mode)
    return _CACHE[mode]


def _run(inputs, trace=False):
    """Returns (full_output [B,T,C], BassKernelResults)."""
    from concourse import bass_utils

    mode, in_maps = _host_prep(
        inputs["x"],
        inputs["prefix_causal_mask"],
        inputs["W_attn"],
        inputs["b_attn"],
        inputs["W_proj"],
    )
    nc = _get_program(mode)
    res = bass_utils.run_bass_kernel_spmd(
        nc, in_maps, core_ids=list(range(NCORES)), trace=trace
    )
    b_proj = np.asarray(inputs["b_proj"], dtype=np.float32)
    y = np.zeros((B, T, C), dtype=np.float32)
    for core in range(NCORES):
        y[core // NHG] += res.results[core]["yp"]
    y += b_proj[None, None, :]
    return y, res


def kernel(**inputs):
    y, _ = _run(inputs, trace=False)
    return y
